# revision 1
# baseline (speedup 1.0000x reference)
"""DynamicConv (attention-over-kernel-bank conv2d) on 8 Trainium2 NeuronCores.

Data-parallel over batch N=32: 4 samples per core. Per core:
  1. pooled mean + tiny MLP + softmax(tau) -> pi [4 samples, 4 mixtures]
  2. per-sample kernel aggregation  aggT[ci, kh, kw, co] = sum_m pi[m] * Wbank
     (DVE scalar_tensor_tensor chain, fp32 accum, bf16 result)
  3. conv2d 3x3 pad 1 as 36 shifted matmuls accumulated in PSUM per
     [co_tile=128 x 512] output block (x padded to 66x66 on host, bf16)
  4. epilogue: + pi @ Bbank.T bias via ScalarE, DMA out fp32.
"""

from contextlib import ExitStack

import ml_dtypes
import numpy as np

import concourse.bass as bass
import concourse.tile as tile
from concourse import bacc, bass_utils, mybir

N, CI, CO, KK, H, W, M = 32, 256, 256, 3, 64, 64, 4
HID = CI // M
TAU = 1.0 / 30.0
NCORES = 8
NL = N // NCORES          # samples per core
CIT, COT = CI // 128, CO // 128
HP = H + 2                # padded spatial
CHUNK_ROWS = 8            # output rows per PSUM block (8*64 = 512 free)
CHUNKS = H // CHUNK_ROWS
TAPS = KK * KK

F32 = mybir.dt.float32
BF16 = mybir.dt.bfloat16
BF16_NP = ml_dtypes.bfloat16

_CACHE: dict = {}


def _emit(ctx: ExitStack, tc: tile.TileContext):
    nc = tc.nc
    AF = mybir.ActivationFunctionType
    ALU = mybir.AluOpType
    AX = mybir.AxisListType

    xpad_d = nc.dram_tensor("xpad", (NL, CIT, 128, HP, HP), BF16, kind="ExternalInput").ap()
    wb_d = nc.dram_tensor("wb", (M, CIT, 128, TAPS, CO), BF16, kind="ExternalInput").ap()
    # one packed f32 blob for all small constants (single DMA trigger):
    # [:, 0:128]  w1t (ci-tile-major, /(H*W) folded)   [128, 2*64]
    # [0:64, 128] b1
    # [0:65, 129:133] w2.T*TAU with b2*TAU appended as row 64
    # [:, 133:141] Bbank.T as [128, COT, M]
    cst_d = nc.dram_tensor("cst", (128, 141), F32, kind="ExternalInput").ap()
    y_d = nc.dram_tensor("y", (NL, COT, 128, CHUNKS, CHUNK_ROWS * W), F32, kind="ExternalOutput").ap()

    consts = ctx.enter_context(tc.tile_pool(name="consts", bufs=1))
    xp_pool = ctx.enter_context(tc.tile_pool(name="xp", bufs=1))
    aggb_pool = ctx.enter_context(tc.tile_pool(name="aggb", bufs=2))
    outp = ctx.enter_context(tc.tile_pool(name="outp", bufs=8))
    cpsum = ctx.enter_context(tc.tile_pool(name="cpsum", bufs=6, space="PSUM"))
    mpsum = ctx.enter_context(tc.tile_pool(name="mpsum", bufs=1, space="PSUM"))

    # ---- DMA issue order == completion order (one spray queue) and each
    # trigger costs ~0.6us on SyncE, so: sample 0's x first, then the packed
    # consts blob, then the kernel bank (ci-tile-major), then remaining x. ----
    # sample 0's x lands as four half-tiles so pooling can start while the
    # rest is still in flight. Issue order interleaves the ci-tiles (t0a,
    # t1a, t0b, t1b) so VectorE (ci-tile 0) and ScalarE (ci-tile 1) both get
    # staggered data instead of one engine waiting for the other's tile.
    xp_sb = xp_pool.tile([128, NL, CIT, HP, HP], BF16)
    HHALF = HP // 2
    for h0, h1 in ((0, HHALF), (HHALF, HP)):
        for t in range(CIT):
            nc.sync.dma_start(xp_sb[:, 0, t, h0:h1], xpad_d[0, t, :, h0:h1])

    cst_sb = consts.tile([128, 141], F32)
    nc.sync.dma_start(cst_sb[:], cst_d[:])
    b1_sb = cst_sb[0:HID, 128:129]
    w2tb_sb = cst_sb[0 : HID + 1, 129:133]

    wb_sb = consts.tile([128, M, CIT, TAPS, CO], BF16)
    for t in range(CIT):
        for m in range(M):
            nc.sync.dma_start(wb_sb[:, m, t], wb_d[m, t])

    for n in range(1, NL):
        for t in range(CIT):
            nc.sync.dma_start(xp_sb[:, n, t], xpad_d[n, t])

    mlp = ctx.enter_context(tc.tile_pool(name="mlp", bufs=2))
    # pooled columns: samples 1-3 use cols 0 (ci-tile 0) and 1 (ci-tile 1);
    # sample 0 uses four partial-sum columns (one per DMA half), combined by
    # extra accumulated MLP matmuls
    pooled = consts.tile([128, 4, NL], F32)
    pi_b = consts.tile([128, NL * M], F32)
    bnT = consts.tile([128, COT, NL], F32)
    prod = consts.tile([128, M], F32)
    pscr = consts.tile([128, HP * HP], BF16)  # ScalarE pooling scratch
    # hmid with a constant-1 row so the logit matmul adds b2*TAU itself
    hmid_sb = consts.tile([HID + 1, 1], F32)
    nc.vector.memset(hmid_sb[HID : HID + 1, :], 1.0)

    # ---- per-sample attention chains, all emitted BEFORE any conv work so
    # the tiny MLP matmuls are not trapped behind a previous sample's 288
    # conv matmuls in the TensorEngine instruction stream. ----
    for n in range(NL):
        s = n * M

        # global average pool (sum; 1/(H*W) folded into w1t host-side).
        # ci-tile 0 on VectorE, ci-tile 1 on the otherwise-idle ScalarE
        # (activation Copy with accum_out) so the two halves run in parallel.
        if n == 0:
            # per-DMA-half partial sums, combined by extra accumulated
            # matmuls below instead of DVE adds
            nc.vector.reduce_sum(pooled[:, 0, n : n + 1], xp_sb[:, n, 0, 0:HHALF], axis=AX.XY)
            nc.vector.reduce_sum(pooled[:, 1, n : n + 1], xp_sb[:, n, 0, HHALF:HP], axis=AX.XY)
            nc.scalar.activation(pscr[:, 0 : HHALF * HP], xp_sb[:, n, 1, 0:HHALF].rearrange("p a b -> p (a b)"), AF.Copy, accum_out=pooled[:, 2, n : n + 1])
            nc.scalar.activation(pscr[:, HHALF * HP : HP * HP], xp_sb[:, n, 1, HHALF:HP].rearrange("p a b -> p (a b)"), AF.Copy, accum_out=pooled[:, 3, n : n + 1])
            cols = [(0, 0), (0, 1), (1, 2), (1, 3)]
        else:
            nc.vector.reduce_sum(pooled[:, 0, n : n + 1], xp_sb[:, n, 0], axis=AX.XY)
            nc.scalar.activation(pscr[:], xp_sb[:, n, 1].rearrange("p a b -> p (a b)"), AF.Copy, accum_out=pooled[:, 1, n : n + 1])
            cols = [(0, 0), (1, 1)]

        # MLP: hmid = relu(pooled @ w1.T + b1) (bias+relu fused on DVE)
        hmid_ps = mpsum.tile([HID, 1], F32)
        for i, (wt, pc) in enumerate(cols):
            nc.tensor.matmul(hmid_ps[:], cst_sb[:, wt * HID : (wt + 1) * HID], pooled[:, pc, n : n + 1], start=(i == 0), stop=(i == len(cols) - 1))
        nc.vector.tensor_scalar(hmid_sb[0:HID, :], hmid_ps[:], b1_sb, 0.0, op0=ALU.add, op1=ALU.max)

        # lt = TAU*logits + TAU*b2 directly from the matmul (constant-1 row);
        # |lt| <= ~0.2, so no max-subtraction needed before exp.
        logit_ps = mpsum.tile([1, M], F32)
        nc.tensor.matmul(logit_ps[:], hmid_sb[:], w2tb_sb, start=True, stop=True)
        pexp = mlp.tile([1, M], F32)
        nc.scalar.activation(pexp[:], logit_ps[:], AF.Exp)
        ssum = mlp.tile([1, 1], F32)
        nc.vector.reduce_sum(ssum[:], pexp[:], axis=AX.X)
        rsum = mlp.tile([1, 1], F32)
        nc.vector.reciprocal(rsum[:], ssum[:])
        pi_n = mlp.tile([1, M], F32)
        nc.vector.tensor_scalar_mul(pi_n[:], pexp[:], rsum[:])

        # broadcast pi row across partitions (source is partition 0)
        nc.gpsimd.partition_broadcast(pi_b[:, s : s + M], pi_n[0:1, :])

        # bias column: bnT[co, n] = sum_m Bbank[co, m] * pi[n, m]
        for ct in range(COT):
            nc.vector.tensor_mul(prod[:], cst_sb[:, 133 + ct * M : 133 + (ct + 1) * M], pi_b[:, s : s + M])
            nc.vector.reduce_sum(bnT[:, ct, n : n + 1], prod[:], axis=AX.X)

    # ---- per-sample: aggregate kernel, conv sweep ----
    aggs = []
    for n in range(NL):
        s = n * M
        # aggregate the per-sample conv kernel; pass granularity is a
        # (ci-tile, co-half) block so the first conv matmuls un-gate after
        # one quarter of the aggregation instead of half.
        acc = aggb_pool.tile([128, CIT, TAPS, CO], BF16, tag="acc", name="acc")
        agg = aggb_pool.tile([128, CIT, TAPS, CO], BF16, tag="agg", name="agg")
        aggs.append(agg)
        def agg_block(t, ch, tap_sl):
            co_sl = slice(ch * 128, (ch + 1) * 128)
            a_o, g_o = acc[:, t, tap_sl, co_sl], agg[:, t, tap_sl, co_sl]
            nc.vector.tensor_scalar_mul(a_o, wb_sb[:, 0, t, tap_sl, co_sl], pi_b[:, s : s + 1])
            nc.vector.scalar_tensor_tensor(a_o, wb_sb[:, 1, t, tap_sl, co_sl], pi_b[:, s + 1 : s + 2], a_o, op0=ALU.mult, op1=ALU.add)
            nc.vector.scalar_tensor_tensor(a_o, wb_sb[:, 2, t, tap_sl, co_sl], pi_b[:, s + 2 : s + 3], a_o, op0=ALU.mult, op1=ALU.add)
            nc.vector.scalar_tensor_tensor(g_o, wb_sb[:, 3, t, tap_sl, co_sl], pi_b[:, s + 3 : s + 4], a_o, op0=ALU.mult, op1=ALU.add)

        for ch in range(COT):
            for t in range(CIT):
                if n == 0 and ch == 0 and t == 0:
                    # the conv-gating block, tap-granular: the first conv
                    # matmul un-gates after one short chain instead of the
                    # whole [9, 128] block
                    for tap in range(TAPS):
                        agg_block(t, ch, slice(tap, tap + 1))
                else:
                    agg_block(t, ch, slice(0, TAPS))

        agg = aggs[n]

        def mm(ps_tile, t, kh, kw, c, ct, start, stop):
            nc.tensor.matmul(
                ps_tile[:],
                agg[:, t, kh * KK + kw, ct * 128 : (ct + 1) * 128],
                xp_sb[:, n, t, c * CHUNK_ROWS + kh : c * CHUNK_ROWS + kh + CHUNK_ROWS, kw : kw + W],
                start=start,
                stop=stop,
            )

        def epilogue(ps_tile, c, ct):
            ot = outp.tile([128, CHUNK_ROWS * W], F32, tag="ot", name="ot")
            nc.vector.tensor_scalar_add(ot[:], ps_tile[:], bnT[:, ct, n : n + 1])
            nc.sync.dma_start(y_d[n, ct, :, c], ot[:])

        for ct in range(COT):
            if n == 0 and ct == 0:
                # Ramp special-case: run tap-half-0 matmuls for 6 chunks while
                # the DVE is still aggregating ci-tile 1 of this sample's
                # kernel, then come back for tap-half-1.
                pss = [cpsum.tile([128, CHUNK_ROWS * W], F32, tag="ps", name="ps") for _ in range(6)]
                for t in range(CIT):
                    for c in range(6):
                        for kh in range(KK):
                            for kw in range(KK):
                                mm(pss[c], t, kh, kw, c, ct,
                                   start=(t == 0 and kh == 0 and kw == 0),
                                   stop=(t == CIT - 1 and kh == KK - 1 and kw == KK - 1))
                for c in range(6):
                    epilogue(pss[c], c, ct)
                rest = range(6, CHUNKS)
            else:
                rest = range(CHUNKS)
            for c in rest:
                if n == NL - 1 and ct == COT - 1 and c == CHUNKS - 1:
                    # the very last chunk: tapered groups (4+2+2 rows) so the
                    # serial kernel-tail epilogue+DMA is quarter-size (earlier
                    # groups drain while PE computes the later ones)
                    for row_off, rows in ((0, 4), (4, 2), (6, 2)):
                        ps = cpsum.tile([128, rows * W], F32, tag="ps", name="ps", padded_shape=[128, CHUNK_ROWS * W])
                        i = 0
                        for t in range(CIT):
                            for kh in range(KK):
                                for kw in range(KK):
                                    r0 = c * CHUNK_ROWS + row_off + kh
                                    nc.tensor.matmul(
                                        ps[:],
                                        agg[:, t, kh * KK + kw, ct * 128 : (ct + 1) * 128],
                                        xp_sb[:, n, t, r0 : r0 + rows, kw : kw + W],
                                        start=(i == 0),
                                        stop=(i == CIT * TAPS - 1),
                                    )
                                    i += 1
                        ot = outp.tile([128, rows * W], F32, tag="ot", name="ot", padded_shape=[128, CHUNK_ROWS * W])
                        nc.vector.tensor_scalar_add(ot[:], ps[:], bnT[:, ct, n : n + 1])
                        nc.sync.dma_start(y_d[n, ct, :, c, row_off * W : (row_off + rows) * W], ot[:])
                    continue
                ps = cpsum.tile([128, CHUNK_ROWS * W], F32, tag="ps", name="ps")
                i = 0
                for t in range(CIT):
                    for kh in range(KK):
                        for kw in range(KK):
                            mm(ps, t, kh, kw, c, ct, start=(i == 0), stop=(i == CIT * TAPS - 1))
                            i += 1
                epilogue(ps, c, ct)


def build_program():
    nc = bacc.Bacc("TRN2", target_bir_lowering=False, debug=False, num_devices=NCORES)
    with tile.TileContext(nc) as tc:
        with ExitStack() as ctx:
            _emit(ctx, tc)
    nc.compile()
    return nc


def prep_inputs(x, Wbank, Bbank, w1, b1, w2, b2):
    """Host-side layout prep. Returns per-core in_maps."""
    x = np.asarray(x, dtype=np.float32)
    Wbank = np.asarray(Wbank, dtype=np.float32)
    x4 = x.reshape(N, CIT, 128, H, W)
    xpad = np.zeros((N, CIT, 128, HP, HP), dtype=BF16_NP)
    xpad[:, :, :, 1 : H + 1, 1 : W + 1] = x4
    wb = np.ascontiguousarray(Wbank.transpose(1, 2, 3, 4, 0)).reshape(M, CIT, 128, TAPS, CO).astype(BF16_NP)
    cst = np.zeros((128, 141), dtype=np.float32)
    # w1t: [128 ci-part, ci-tile * 64 hid], 1/(H*W) folded
    w1t = (np.asarray(w1, dtype=np.float32) / float(H * W)).T.reshape(CIT, 128, HID)
    for t in range(CIT):
        cst[:, t * HID : (t + 1) * HID] = w1t[t]
    cst[0:HID, 128] = np.asarray(b1, dtype=np.float32)
    cst[0:HID, 129:133] = np.asarray(w2, dtype=np.float32).T * TAU
    cst[HID, 129:133] = np.asarray(b2, dtype=np.float32) * TAU
    cst[:, 133:141] = np.asarray(Bbank, dtype=np.float32).reshape(COT, 128, M).transpose(1, 0, 2).reshape(128, COT * M)
    shared = {"wb": wb, "cst": cst}
    return [{"xpad": np.ascontiguousarray(xpad[c * NL : (c + 1) * NL]), **shared} for c in range(NCORES)]


def kernel(x, Wbank, Bbank, w1, b1, w2, b2):
    x = np.asarray(x, dtype=np.float32)
    in_maps = prep_inputs(x, Wbank, Bbank, w1, b1, w2, b2)
    if "nc" not in _CACHE:
        _CACHE["nc"] = build_program()
    res = bass_utils.run_bass_kernel_spmd(_CACHE["nc"], in_maps, core_ids=list(range(NCORES)))
    return np.concatenate([r["y"].reshape(NL, CO, H, W) for r in res.results], axis=0)

